# revision 28
# baseline (speedup 1.0000x reference)
"""ODE-RNN encoder (GRU-ODE scan) Trainium2 Bass kernel.

Strategy (data-parallel over trajectories):
  - 4096 trajectories sharded 512/core over 8 NeuronCores; all weights
    replicated. The T=128 time scan runs locally per core, no cross-core
    communication. Host gathers the per-core z0 outputs at the end.
  - On-chip layout is feature-on-partition, batch-on-free-dim. Each core's
    512-batch is split into 2 dephased chunks of 256 so the serial
    per-step dependency chain of one chunk hides under engine work of the
    other.
  - Matmuls run as float32r (full PE rate at N>=256), weights stationary in
    SBUF for all 128 steps. PSUM accumulation implements the ODE Euler step
    (y + dt*mlp via identity-matmul accumulate, dt folded into a scaled
    copy of ode_w2 per distinct dt value).
  - Gate algebra is restructured to minimize vector work:
      v = 1-u = sigmoid(-mlp_u)   (negated+duplicated ug_w2 -> [128] rows)
      r2 = sigmoid(mlp_r)         (duplicated rg_w2 -> [128] rows)
      state' = S + (m*v) * (ns' - S)   with S = [y_ode; s] stacked [128,B]
    The trailing abs of the reference is a provable no-op (s stays >= 0).
  - ACT ops fused pairwise (ug|rg tanh, v|r sigmoid share one PSUM bank);
    mask multiplies run on GPSIMD; mask duplication via SBUF->SBUF DMA.

kernel(**inputs) takes the full unsharded numpy inputs (as produced by the
reference setup) and returns (z0_mu, z0_std), each (1, 4096, 64) float32.
"""

import os
import sys

import numpy as np

N_TRAJ = 4096
T = 128
LAT = 64
NDATA = 64
INP = 2 * NDATA
NGRU = 100
NODE = 100
TZ = 100
NCORES = 8
B = N_TRAJ // NCORES          # 512 per core
CH = 2                        # chunks per core
BC = B // CH                  # 256 batch per chunk

_cache = {}


def _build(dts, use_bias):
    import concourse.bass as bass
    import concourse.tile as tile
    from concourse import bacc, mybir

    uniq = list(dict.fromkeys(dts))
    dt_idx = [uniq.index(d) for d in dts]
    n_dt = len(uniq)

    f32 = mybir.dt.float32
    f32r = mybir.dt.float32r
    ACT = mybir.ActivationFunctionType

    nc = bacc.Bacc("TRN2", target_bir_lowering=False, debug=False,
                   num_devices=NCORES)

    # ---- DRAM I/O ----
    xT_d = nc.dram_tensor("xT", [T, INP, B], f32r, kind="ExternalInput")
    wug1_d = nc.dram_tensor("wug1", [2 * LAT + INP, NGRU], f32r, kind="ExternalInput")
    wrg1_d = nc.dram_tensor("wrg1", [2 * LAT + INP, NGRU], f32r, kind="ExternalInput")
    wns1_d = nc.dram_tensor("wns1", [2 * LAT + INP, NGRU], f32r, kind="ExternalInput")
    wug2_d = nc.dram_tensor("wug2nd", [NGRU, 2 * LAT], f32r, kind="ExternalInput")
    wrg2_d = nc.dram_tensor("wrg2d", [NGRU, 2 * LAT], f32r, kind="ExternalInput")
    wns2_d = nc.dram_tensor("wns2", [NGRU, 2 * LAT], f32r, kind="ExternalInput")
    wode1_d = nc.dram_tensor("wode1", [LAT, NODE], f32r, kind="ExternalInput")
    wode2_d = nc.dram_tensor("wode2", [NODE, LAT], f32r, kind="ExternalInput")
    wfug_d = nc.dram_tensor("wfuse_ug", [n_dt, NODE, NGRU], f32r, kind="ExternalInput")
    wfrg_d = nc.dram_tensor("wfuse_rg", [n_dt, NODE, NGRU], f32r, kind="ExternalInput")
    negI_d = nc.dram_tensor("negI128", [2 * LAT, 2 * LAT], f32r, kind="ExternalInput")
    wtz1_d = nc.dram_tensor("wtz1", [2 * LAT, TZ], f32r, kind="ExternalInput")
    wtz2_d = nc.dram_tensor("wtz2", [TZ, 2 * LAT], f32r, kind="ExternalInput")
    if use_bias:
        bode1_d = nc.dram_tensor("bode1", [NODE, 1], f32, kind="ExternalInput")
        bns1_d = nc.dram_tensor("bns1", [NGRU, 1], f32, kind="ExternalInput")
        bns2b_d = nc.dram_tensor("bns2b", [LAT, 1], f32, kind="ExternalInput")
        btz1_d = nc.dram_tensor("btz1", [TZ, 1], f32, kind="ExternalInput")
        btz2t_d = nc.dram_tensor("btz2t", [LAT, 1], f32, kind="ExternalInput")
        btz2b_d = nc.dram_tensor("btz2b", [LAT, 1], f32, kind="ExternalInput")
        # row-vector biases (K=1 matmul accumulate): [1, M]
        bug1_d = nc.dram_tensor("bug1r", [1, NGRU], f32r, kind="ExternalInput")
        brg1_d = nc.dram_tensor("brg1r", [1, NGRU], f32r, kind="ExternalInput")
        bug2_d = nc.dram_tensor("bug2ndr", [1, 2 * LAT], f32r, kind="ExternalInput")
        brg2_d = nc.dram_tensor("brg2dr", [1, 2 * LAT], f32r, kind="ExternalInput")
        bns2t_d = nc.dram_tensor("bns2tr", [1, LAT], f32r, kind="ExternalInput")
        bode2_d = nc.dram_tensor("bode2r", [1, LAT], f32r, kind="ExternalInput")
        ones_d = nc.dram_tensor("ones1", [1, BC], f32r, kind="ExternalInput")
    zeros_d = nc.dram_tensor("zeros0", [2 * LAT, B], f32r, kind="ExternalInput")
    zout_d = nc.dram_tensor("zout", [2 * LAT, B], f32, kind="ExternalOutput")

    def r(ap):
        return ap.bitcast(f32r)

    def c32(ap):
        return ap.bitcast(f32)

    with tile.TileContext(nc) as tc:
        with (
            tc.tile_pool(name="const", bufs=1) as cpool,
            tc.tile_pool(name="state", bufs=1) as spool,
            tc.tile_pool(name="xin", bufs=3) as xpool,
            tc.tile_pool(name="mdup", bufs=2) as mpool,
            tc.tile_pool(name="tmp0", bufs=2) as tpool0,
            tc.tile_pool(name="tmp1", bufs=2) as tpool1,
            tc.tile_pool(name="psA0", bufs=1, space="PSUM") as psA0,
            tc.tile_pool(name="psB0", bufs=1, space="PSUM") as psB0,
            tc.tile_pool(name="g1p0", bufs=1, space="PSUM") as g1p0,
            tc.tile_pool(name="g2p0", bufs=1, space="PSUM") as g2p0,
            tc.tile_pool(name="psA1", bufs=1, space="PSUM") as psA1,
            tc.tile_pool(name="psB1", bufs=1, space="PSUM") as psB1,
            tc.tile_pool(name="g1p1", bufs=1, space="PSUM") as g1p1,
            tc.tile_pool(name="g2p1", bufs=1, space="PSUM") as g2p1,
        ):
            tpool = [tpool0, tpool1]
            psA = [psA0, psA1]
            psB = [psB0, psB1]
            g1p = [g1p0, g1p1]
            g2p = [g2p0, g2p1]

            # ---- load constants ----
            def cload(shape, src_ap, tag, dt_=None):
                t = cpool.tile(shape, dt_ or f32r, tag=tag, name=tag)
                nc.sync.dma_start(t[:, :], src_ap)
                return t

            wug1a = cload([INP, NGRU], wug1_d[0:INP, :], "wug1a")
            wug1b = cload([INP, NGRU], wug1_d[INP:2 * LAT + INP, :], "wug1b")
            wrg1a = cload([INP, NGRU], wrg1_d[0:INP, :], "wrg1a")
            wrg1b = cload([INP, NGRU], wrg1_d[INP:2 * LAT + INP, :], "wrg1b")
            wns1a = cload([INP, NGRU], wns1_d[0:INP, :], "wns1a")
            wns1b = cload([INP, NGRU], wns1_d[INP:2 * LAT + INP, :], "wns1b")
            # note: rows 0:128 of w*1 multiply [y;s] (=128 rows), rows 128:256
            # multiply x (=128 rows); INP == 2*LAT == 128 here.
            wug2 = cload([NGRU, 2 * LAT], wug2_d[:, :], "wug2")
            wrg2 = cload([NGRU, 2 * LAT], wrg2_d[:, :], "wrg2")
            wns2 = cload([NGRU, 2 * LAT], wns2_d[:, :], "wns2")
            wode1 = cload([LAT, NODE], wode1_d[:, :], "wode1")
            wtz1 = cload([2 * LAT, TZ], wtz1_d[:, :], "wtz1")
            wtz2 = cload([TZ, 2 * LAT], wtz2_d[:, :], "wtz2")
            wode2 = cload([NODE, LAT], wode2_d[:, :], "wode2")
            wfug = [cload([NODE, NGRU], wfug_d[i], f"wfug{i}") for i in range(n_dt)]
            wfrg = [cload([NODE, NGRU], wfrg_d[i], f"wfrg{i}") for i in range(n_dt)]
            negI = cload([2 * LAT, 2 * LAT], negI_d[:, :], "negI")
            if use_bias:
                bode1 = cload([NODE, 1], bode1_d[:, :], "bode1", f32)
                bns1 = cload([NGRU, 1], bns1_d[:, :], "bns1", f32)
                btz1 = cload([TZ, 1], btz1_d[:, :], "btz1", f32)
                btz2t = cload([LAT, 1], btz2t_d[:, :], "btz2t", f32)
                # biases applied on partitions 64:128 must live there too
                bns2b = cpool.tile([2 * LAT, 1], f32, tag="bns2b", name="bns2b")
                nc.sync.dma_start(bns2b[LAT:2 * LAT, :], bns2b_d[:, :])
                btz2b = cpool.tile([2 * LAT, 1], f32, tag="btz2b", name="btz2b")
                nc.sync.dma_start(btz2b[LAT:2 * LAT, :], btz2b_d[:, :])
                bug1r = cload([1, NGRU], bug1_d[:, :], "bug1r")
                brg1r = cload([1, NGRU], brg1_d[:, :], "brg1r")
                bug2r = cload([1, 2 * LAT], bug2_d[:, :], "bug2r")
                brg2r = cload([1, 2 * LAT], brg2_d[:, :], "brg2r")
                bns2tr = cload([1, LAT], bns2t_d[:, :], "bns2tr")
                bode2r = cload([1, LAT], bode2_d[:, :], "bode2r")
                ones = cpool.tile([1, BC], f32r, tag="ones", name="ones")
                nc.sync.dma_start(ones[:, :], ones_d[:, :])

            def b_act(t):  # ACT bias operand (or 0.0 when biases disabled)
                return t[:, :] if use_bias else 0.0

            # ---- state tiles (ping-pong per chunk) ----
            S = [[spool.tile([2 * LAT, BC], f32r, tag=f"s{c}_{p}",
                             name=f"s{c}_{p}")
                  for p in range(2)] for c in range(CH)]
            for c in range(CH):
                nc.sync.dma_start(S[c][0][:, :],
                                  zeros_d[:, c * BC:(c + 1) * BC])

            # ---- the scan ----
            # Software-pipelined emission: chunk 1 runs OFF stages behind
            # chunk 0 in program order, so each engine's in-order queue always
            # has the other chunk's (independent) work behind the current op
            # and chain-handoff stalls of the two chunks never coincide.
            def new_ctx(c, t):
                return dict(cs=slice(c * BC, (c + 1) * BC),
                            Sc=S[c][t % 2], Sn=S[c][(t + 1) % 2],
                            tp=tpool[c], t=t)

            def s_ode1(c, d, xt, m2):
                d['ps_oh'] = psA[c].tile([NODE, BC], f32, tag="psA",
                                         name=f"oh{c}")
                d['ode1'] = nc.tensor.matmul(d['ps_oh'][:, :], r(wode1[:, :]),
                                             r(d['Sc'][0:LAT, :]),
                                             start=True, stop=True)

            def s_tanh_ode(c, d, xt, m2):
                d['h_ode'] = d['tp'].tile([NODE, BC], f32r, tag="h_ode",
                                          name=f"ho{c}")
                nc.scalar.activation(d['h_ode'][:, :], d['ps_oh'][:, :],
                                     ACT.Tanh,
                                     bias=b_act(bode1) if use_bias else 0.0)

            def s_ode2(c, d, xt, m2):
                d['ps_yo'] = psB[c].tile([LAT, BC], f32, tag="psB",
                                         name=f"yo{c}")
                nc.tensor.matmul(d['ps_yo'][:, :], r(wode2[:, :]),
                                 r(d['h_ode'][:, :]), start=True,
                                 stop=not use_bias)
                if use_bias:
                    nc.tensor.matmul(d['ps_yo'][:, :], r(bode2r[:, :]),
                                     r(ones[:, :]), start=False, stop=True)

            def s_yode(c, d, xt, m2):
                nc.vector.scalar_tensor_tensor(
                    d['Sc'][0:LAT, :], d['ps_yo'][:, :], float(dts[d['t']]),
                    c32(d['Sc'][0:LAT, :]),
                    op0=mybir.AluOpType.mult, op1=mybir.AluOpType.add)

            def s_ug1(c, d, xt, m2):
                g1 = d['g1']
                nc.tensor.matmul(g1[:, 0:BC], r(wug1b[:, :]),
                                 r(xt[:, d['cs']]), start=True, stop=False)
                if use_bias:
                    nc.tensor.matmul(g1[:, 0:BC], r(bug1r[:, :]),
                                     r(ones[:, :]), start=False, stop=False)
                nc.tensor.matmul(g1[:, 0:BC], r(wug1a[:, :]),
                                 r(d['Sc'][:, :]), start=False, stop=False)

            def s_ug1c(c, d, xt, m2):
                nc.tensor.matmul(d['g1'][:, 0:BC],
                                 r(wfug[dt_idx[d['t']]][:, :]),
                                 r(d['h_ode'][:, :]), start=False, stop=True)

            def s_rg1(c, d, xt, m2):
                d['g1'] = g1p[c].tile([NGRU, 2 * BC], f32, tag="g1",
                                      name=f"g1_{c}")
                g1 = d['g1']
                nc.tensor.matmul(g1[:, BC:2 * BC], r(wrg1b[:, :]),
                                 r(xt[:, d['cs']]), start=True, stop=False)
                if use_bias:
                    nc.tensor.matmul(g1[:, BC:2 * BC], r(brg1r[:, :]),
                                     r(ones[:, :]), start=False, stop=False)
                nc.tensor.matmul(g1[:, BC:2 * BC], r(wrg1a[:, :]),
                                 r(d['Sc'][:, :]), start=False, stop=False)

            def s_rg1c(c, d, xt, m2):
                # fused ODE correction: dt*W_rg1[y]^T(wode2^T h_ode)
                nc.tensor.matmul(d['g1'][:, BC:2 * BC],
                                 r(wfrg[dt_idx[d['t']]][:, :]),
                                 r(d['h_ode'][:, :]), start=False, stop=True)

            def s_tanh_r(c, d, xt, m2):
                d['h_g'] = d['tp'].tile([NGRU, 2 * BC], f32r, tag="h_g",
                                        name=f"hg{c}")
                nc.scalar.activation(d['h_g'][:, BC:2 * BC],
                                     d['g1'][:, BC:2 * BC], ACT.Tanh)

            def s_tanh_u(c, d, xt, m2):
                nc.scalar.activation(d['h_g'][:, 0:BC], d['g1'][:, 0:BC],
                                     ACT.Tanh)

            def s_rg2(c, d, xt, m2):
                d['g2'] = g2p[c].tile([2 * LAT, 2 * BC], f32, tag="g2",
                                      name=f"g2_{c}")
                nc.tensor.matmul(d['g2'][:, BC:2 * BC], r(wrg2[:, :]),
                                 r(d['h_g'][:, BC:2 * BC]),
                                 start=True, stop=use_bias is False)
                if use_bias:
                    nc.tensor.matmul(d['g2'][:, BC:2 * BC], r(brg2r[:, :]),
                                     r(ones[:, :]), start=False, stop=True)

            def s_sig_r(c, d, xt, m2):
                d['vr'] = d['tp'].tile([2 * LAT, 2 * BC], f32, tag="vr",
                                       name=f"vr{c}")
                d['sig_r'] = nc.scalar.activation(d['vr'][:, BC:2 * BC],
                                                  d['g2'][:, BC:2 * BC],
                                                  ACT.Sigmoid)

            def s_ug2(c, d, xt, m2):
                nc.tensor.matmul(d['g2'][:, 0:BC], r(wug2[:, :]),
                                 r(d['h_g'][:, 0:BC]),
                                 start=True, stop=use_bias is False)
                if use_bias:
                    nc.tensor.matmul(d['g2'][:, 0:BC], r(bug2r[:, :]),
                                     r(ones[:, :]), start=False, stop=True)

            def s_sig_v(c, d, xt, m2):
                nc.scalar.activation(d['vr'][:, 0:BC], d['g2'][:, 0:BC],
                                     ACT.Sigmoid)

            def s_gm(c, d, xt, m2):
                d['g'] = d['tp'].tile([2 * LAT, BC], f32, tag="g",
                                      name=f"g{c}")
                nc.vector.tensor_mul(d['g'][:, :], c32(m2[:, d['cs']]),
                                     d['vr'][:, 0:BC])

            def s_ryc(c, d, xt, m2):
                d['ryc'] = d['tp'].tile([2 * LAT, BC], f32r, tag="ryc",
                                        name=f"ryc{c}")
                nc.vector.tensor_mul(d['ryc'][:, :], d['vr'][:, BC:2 * BC],
                                     c32(d['Sc'][:, :]))

            def s_ns1(c, d, xt, m2):
                d['n1'] = psA[c].tile([NGRU, BC], f32, tag="psA",
                                      name=f"n1_{c}")
                nc.tensor.matmul(d['n1'][:, :], r(wns1b[:, :]),
                                 r(xt[:, d['cs']]), start=True, stop=False)
                nc.tensor.matmul(d['n1'][:, :], r(wns1a[:, :]),
                                 r(d['ryc'][:, :]), start=False, stop=True)

            def s_tanh_ns(c, d, xt, m2):
                d['h_n'] = d['tp'].tile([NGRU, BC], f32r, tag="h_n",
                                        name=f"hn{c}")
                nc.scalar.activation(d['h_n'][:, :], d['n1'][:, :], ACT.Tanh,
                                     bias=b_act(bns1) if use_bias else 0.0)

            def s_ns2(c, d, xt, m2):
                d['n2'] = psB[c].tile([2 * LAT, BC], f32, tag="psB",
                                      name=f"n2_{c}")
                nc.tensor.matmul(d['n2'][:, :], r(wns2[:, :]),
                                 r(d['h_n'][:, :]), start=True, stop=False)
                if use_bias:
                    nc.tensor.matmul(d['n2'][0:LAT, :], r(bns2tr[:, :]),
                                     r(ones[:, :]), start=False, stop=False)

            def s_negI_top(c, d, xt, m2):
                nc.tensor.matmul(d['n2'][0:LAT, :], r(negI[0:LAT, 0:LAT]),
                                 r(d['Sc'][0:LAT, :]), start=False, stop=True,
                                 skip_group_check=True)

            def s_abs(c, d, xt, m2):
                n2 = d['n2']
                nc.scalar.activation(n2[LAT:2 * LAT, :], n2[LAT:2 * LAT, :],
                                     ACT.Abs,
                                     bias=bns2b[LAT:2 * LAT, :] if use_bias else 0.0)

            # blend is split: the y-half is on the serial chain (next step's
            # ODE needs y only); the s-half trails behind abs with slack.
            def s_negI_bot(c, d, xt, m2):
                # DVE: q_bot = |ns_b| - s  (fp32 matmul cannot write PSUM at
                # a partition offset, so the s-half subtract runs on DVE)
                d['qb'] = d['tp'].tile([2 * LAT, BC], f32, tag="qb",
                                       name=f"qb{c}")
                nc.vector.tensor_sub(d['qb'][LAT:2 * LAT, :],
                                     d['n2'][LAT:2 * LAT, :],
                                     c32(d['Sc'][LAT:2 * LAT, :]))

            def s_gtq_top(c, d, xt, m2):
                d['gtq'] = d['tp'].tile([2 * LAT, BC], f32r, tag="gtq",
                                        name=f"gtq{c}")
                nc.vector.tensor_mul(d['gtq'][0:LAT, :], d['g'][0:LAT, :],
                                     d['n2'][0:LAT, :])

            def s_gtq_bot(c, d, xt, m2):
                nc.vector.tensor_mul(d['gtq'][LAT:2 * LAT, :],
                                     d['g'][LAT:2 * LAT, :],
                                     d['qb'][LAT:2 * LAT, :])

            def s_add_top(c, d, xt, m2):
                nc.vector.tensor_add(d['Sn'][0:LAT, :],
                                     c32(d['Sc'][0:LAT, :]),
                                     c32(d['gtq'][0:LAT, :]))

            def s_add_bot(c, d, xt, m2):
                nc.vector.tensor_add(d['Sn'][LAT:2 * LAT, :],
                                     c32(d['Sc'][LAT:2 * LAT, :]),
                                     c32(d['gtq'][LAT:2 * LAT, :]))

            from concourse.tile import add_dep_helper

            stages = [s_ode1, s_tanh_ode, s_rg1, s_rg1c, s_tanh_r, s_ug1,
                      s_rg2, s_sig_r, s_ode2, s_yode, s_ryc, s_ug1c, s_tanh_u,
                      s_ug2, s_ns1, s_sig_v, s_tanh_ns, s_gm, s_ns2,
                      s_negI_top, s_gtq_top, s_add_top, s_abs, s_negI_bot,
                      s_gtq_bot, s_add_bot]
            NS = len(stages)
            SIG_IDX = stages.index(s_sig_r)
            OFF = 17  # align the other chunk's PE-free blend tail over
            # this chunk's chain-matmul handoff window (reduces PE queue
            # collisions on the serial chain)
            total = T * NS
            ctx = [None, None]
            xts = {}
            last_sig = [None, None]   # most recent sigma_r instruction per chunk
            for n in range(total + OFF):
                if n < total:
                    t, k = divmod(n, NS)
                    if k == 0:
                        xt = xpool.tile([INP, B], f32r, tag="xt",
                                        name=f"xt{t % 4}")
                        nc.sync.dma_start(xt[:, :], xT_d[t])
                        m2 = mpool.tile([INP, B], f32r, tag="m2",
                                        name=f"m2_{t % 4}")
                        nc.gpsimd.dma_start(m2[0:NDATA, :], xt[NDATA:INP, :])
                        nc.gpsimd.dma_start(m2[NDATA:INP, :], xt[NDATA:INP, :])
                        xts[t] = (xt, m2)
                        ctx[0] = new_ctx(0, t)
                    stages[k](0, ctx[0], *xts[t])
                    if k == 0 and last_sig[1] is not None:
                        # anti-phase: chunk0's next step starts no earlier
                        # than chunk1's reset-gate sigmoid of its prior step
                        add_dep_helper(ctx[0]['ode1'].ins, last_sig[1].ins, sync=False,
                                       reason="anti-phase c0<-c1")
                    if k == SIG_IDX:
                        last_sig[0] = ctx[0].get('sig_r')
                m = n - OFF
                if m >= 0:
                    t, k = divmod(m, NS)
                    if k == 0:
                        ctx[1] = new_ctx(1, t)
                    stages[k](1, ctx[1], *xts[t])
                    if k == 0 and last_sig[0] is not None:
                        add_dep_helper(ctx[1]['ode1'].ins, last_sig[0].ins, sync=False,
                                       reason="anti-phase c1<-c0")
                    if k == SIG_IDX:
                        last_sig[1] = ctx[1].get('sig_r')
                    if k == NS - 1:
                        xts.pop(t)

            # ---- final transform z0 = mlp2([y; s]) ----
            for c in range(CH):
                cs = slice(c * BC, (c + 1) * BC)
                Sf = S[c][T % 2]
                pt1 = psA[c].tile([TZ, BC], f32, tag="psA")
                nc.tensor.matmul(pt1[:, :], r(wtz1[:, :]), r(Sf[:, :]),
                                 start=True, stop=True)
                h_t = tpool[c].tile([TZ, BC], f32r, tag="h_t")
                nc.scalar.activation(h_t[:, :], pt1[:, :], ACT.Tanh,
                                     bias=b_act(btz1) if use_bias else 0.0)
                pt2 = psB[c].tile([2 * LAT, BC], f32, tag="psB")
                nc.tensor.matmul(pt2[:, :], r(wtz2[:, :]), r(h_t[:, :]),
                                 start=True, stop=True)
                zo = tpool[c].tile([2 * LAT, BC], f32, tag="zo")
                nc.scalar.activation(zo[0:LAT, :], pt2[0:LAT, :], ACT.Copy,
                                     bias=b_act(btz2t) if use_bias else 0.0)
                nc.scalar.activation(zo[LAT:2 * LAT, :], pt2[LAT:2 * LAT, :],
                                     ACT.Abs,
                                     bias=btz2b[LAT:2 * LAT, :] if use_bias else 0.0)
                nc.sync.dma_start(zout_d[:, cs], zo[:, :])

    nc.compile()
    return nc


def _prep(inputs):
    g = lambda k: np.ascontiguousarray(np.asarray(inputs[k], dtype=np.float32))
    data = g("data")
    tps = g("tps")
    W = {k: g(k) for k in (
        "ug_w1", "ug_b1", "ug_w2", "ug_b2", "rg_w1", "rg_b1", "rg_w2", "rg_b2",
        "ns_w1", "ns_b1", "ns_w2", "ns_b2", "ode_w1", "ode_b1", "ode_w2",
        "ode_b2", "tz_w1", "tz_b1", "tz_w2", "tz_b2")}

    rev = tps[::-1]
    dts = np.concatenate([np.full((1,), -0.01, np.float32),
                          rev[1:] - rev[:-1]]).astype(np.float32)
    dts = tuple(float(d) for d in dts.tolist())

    use_bias = any(float(np.abs(W[k]).max()) != 0.0 for k in W if "_b" in k)

    # time-reverse + transpose: [T, INP, N_TRAJ], contiguous
    xT_full = np.ascontiguousarray(data[:, ::-1, :].transpose(1, 2, 0))

    uniq = list(dict.fromkeys(dts))
    common = {
        "wfuse_ug": np.stack([np.float32(d) * (W["ode_w2"] @ W["ug_w1"][:LAT])
                              for d in uniq]),
        "wfuse_rg": np.stack([np.float32(d) * (W["ode_w2"] @ W["rg_w1"][:LAT])
                              for d in uniq]),
        "wug1": W["ug_w1"],
        "wrg1": W["rg_w1"],
        "wns1": W["ns_w1"],
        "wug2nd": -np.concatenate([W["ug_w2"], W["ug_w2"]], axis=1),
        "wrg2d": np.concatenate([W["rg_w2"], W["rg_w2"]], axis=1),
        "wns2": W["ns_w2"],
        "wode1": W["ode_w1"],
        "wode2": W["ode_w2"],
        "negI128": -np.eye(2 * LAT, dtype=np.float32),
        "wtz1": W["tz_w1"],
        "wtz2": W["tz_w2"],
        "zeros0": np.zeros((2 * LAT, B), np.float32),
    }
    if use_bias:
        col = lambda v: np.ascontiguousarray(v.reshape(-1, 1))
        row = lambda v: np.ascontiguousarray(v.reshape(1, -1))
        common.update({
            "bode1": col(W["ode_b1"]),
            "bns1": col(W["ns_b1"]),
            "bns2b": col(W["ns_b2"][LAT:]),
            "btz1": col(W["tz_b1"]),
            "btz2t": col(W["tz_b2"][:LAT]),
            "btz2b": col(W["tz_b2"][LAT:]),
            "bug1r": row(W["ug_b1"] + W["ug_w1"][:LAT].T @ (W["ode_b2"] * 0)),
            "brg1r": row(W["rg_b1"] + W["rg_w1"][:LAT].T @ (W["ode_b2"] * 0)),
            "bug2ndr": row(-np.concatenate([W["ug_b2"], W["ug_b2"]])),
            "brg2dr": row(np.concatenate([W["rg_b2"], W["rg_b2"]])),
            "bns2tr": row(W["ns_b2"][:LAT]),
            "bode2r": row(W["ode_b2"]),
            "ones1": np.ones((1, BC), np.float32),
        })
    common = {k: np.ascontiguousarray(v.astype(np.float32))
              for k, v in common.items()}

    in_maps = []
    for c in range(NCORES):
        m = dict(common)
        m["xT"] = np.ascontiguousarray(xT_full[:, :, c * B:(c + 1) * B])
        in_maps.append(m)
    return in_maps, dts, use_bias


def _ensure_ntff_hook():
    """run_bass_kernel_spmd(trace=True) under axon imports
    antenv.axon_hooks, which is absent in this image. Install a stub so a
    BASS_TRACE=1 environment cannot crash the run; the stub returns the
    real ctypes-based profiler hook when available, else None (bass_utils
    then skips tracing gracefully)."""
    import types as _types
    if "antenv.axon_hooks" in sys.modules:
        return
    hook = None
    try:
        from trn_agent_boot.trn_boot import _ntff_profile_via_ctypes
        hook = _ntff_profile_via_ctypes("/opt/axon/libaxon_pjrt.so")
    except Exception:
        hook = None
    try:
        import antenv
        mod = _types.ModuleType("antenv.axon_hooks")
        mod.get_axon_ntff_profile_hook = lambda: hook
        mod.set_axon_ntff_profile_hook = lambda h: None
        sys.modules["antenv.axon_hooks"] = mod
        antenv.axon_hooks = mod
    except Exception:
        pass


def _run(inputs, trace=False, trace_kwargs=None):
    _ensure_ntff_hook()
    from concourse.bass_utils import run_bass_kernel_spmd

    in_maps, dts, use_bias = _prep(inputs)
    key = (dts, use_bias)
    if key not in _cache:
        _cache[key] = _build(dts, use_bias)
    nc = _cache[key]

    res = run_bass_kernel_spmd(nc, in_maps, list(range(NCORES)),
                               trace=trace, **(trace_kwargs or {}))
    mu = np.empty((N_TRAJ, LAT), np.float32)
    std = np.empty((N_TRAJ, LAT), np.float32)
    for c in range(NCORES):
        z = res.results[c]["zout"]
        mu[c * B:(c + 1) * B] = z[0:LAT].T
        std[c * B:(c + 1) * B] = z[LAT:2 * LAT].T
    return (mu[None], std[None]), res


def kernel(**inputs):
    out, _ = _run(inputs, trace=False)
    return out
